# revision 1
# baseline (speedup 1.0000x reference)
"""BottleneckMamba Trainium2 kernel (self-contained).

out = x + cv2( scale * out_proj( LN(cross-merge(4-dir selective scan(N=1))) * z ) )

3 SPMD launches on 8 NeuronCores:
  L1 (core=(b, image-half)): cv1 -> h; depthwise3x3*in_proj folded into 9
     matmuls -> silu -> xc ; z = silu(Wz@h) ; B/C projection rows.
  L2 (core=(b, dir-group)): g=0 dirs {0,2} on xc row-major; g=1 dirs {1,3} on
     host-transposed xc (identical program). Per dir: dtd matmul ->
     exp/ln1p/exp on ACT (softplus+decay, one table set) -> dt*u*Bbc ->
     tensor_tensor_scan (reversed-AP traversal for the backward dir) ->
     h*Cbc ; PE merges the pair + D*u -> m.
  L3 (core=(b, half)): y = m02 + m13^T (host transposes m13), LayerNorm via
     matmul stats + rank-1 broadcast matmuls, *z, fused
     (cv2 @ diag(scale) @ out_proj) matmul + bias -> delta.
Host: shards/reassembles, transposes between launches, adds residual x.
"""
import os
import sys

sys.path.insert(0, '/opt/trn_rl_repo')

import numpy as np
import ml_dtypes

import concourse.bass as bass
import concourse.tile as tile
import concourse.mybir as mybir
from concourse.bass_utils import run_bass_kernel_spmd

bf16 = mybir.dt.bfloat16
f32 = mybir.dt.float32
MULT, ADD = mybir.AluOpType.mult, mybir.AluOpType.add
SUB = mybir.AluOpType.subtract
AF = mybir.ActivationFunctionType
NBF = ml_dtypes.bfloat16

B, C1, C2, H, W = 4, 256, 256, 128, 128
Cm, K, R = 128, 4, 8
L = H * W          # 16384
HH = H // 2        # 64 rows per half
LH = HH * W        # 8192
CH = 2048          # L2/L3 chunk
NCH = L // CH      # 8

EXEC_TIMES = {}    # launch -> exec ns (MAMBA_TRACE=1)
_CACHE = {}


def _split_multiwaits(nc):
    """walrus here accepts ONE sync-wait per instruction; hoist extras into
    single-wait same-engine NOPs inserted before the instruction."""
    for f in nc.m.functions:
        for bb in f.blocks:
            il = bb.instructions
            i = 0
            while i < len(il):
                ins = il[i]
                si = getattr(ins, "sync_info", None)
                if si is not None and len(si.on_wait) > 1:
                    waits = list(si.on_wait)
                    ins.sync_info = mybir.SyncInfo(
                        on_wait=[waits[-1]], on_update=list(si.on_update))
                    for w in waits[:-1]:
                        nop = mybir.InstNoOp(
                            name=nc.get_next_instruction_name(), ins=[], outs=[])
                        nop.engine = ins.engine
                        nop.sync_info = mybir.SyncInfo(on_wait=[w], on_update=[])
                        nc.register_instruction(nop, overwrite=True)
                        il.insert(i, nop)
                        i += 1
                i += 1


def _new_nc():
    return bass.Bass("TRN2", target_bir_lowering=False, debug=False,
                     enable_asserts=True, num_devices=8)


def _run(nc, in_maps, name):
    trace = os.environ.get("MAMBA_TRACE", "0") == "1"
    res = run_bass_kernel_spmd(nc, in_maps, core_ids=list(range(8)), trace=trace)
    if trace:
        EXEC_TIMES[name] = res.exec_time_ns
    return res.results


# ------------------------------------------------------------------- L1
def build_l1():
    nc = _new_nc()
    x_in = nc.dram_tensor("x_in", [C1, HH + 2, W], f32, kind="ExternalInput")
    wcv1 = nc.dram_tensor("wcv1", [C1, Cm], f32, kind="ExternalInput")       # lhsT
    bcv1 = nc.dram_tensor("bcv1", [Cm, 1], f32, kind="ExternalInput")
    wfold = nc.dram_tensor("wfold", [Cm, 9, Cm], f32, kind="ExternalInput")  # (k, tap, m)
    bconv = nc.dram_tensor("bconv", [Cm, 1], f32, kind="ExternalInput")
    wz = nc.dram_tensor("wz", [Cm, Cm], f32, kind="ExternalInput")           # lhsT
    wbc = nc.dram_tensor("wbc", [Cm, 8], f32, kind="ExternalInput")          # lhsT
    hmask = nc.dram_tensor("hmask", [Cm, 2], f32, kind="ExternalInput")
    xc_out = nc.dram_tensor("xc_out", [Cm, LH], bf16, kind="ExternalOutput")
    z_out = nc.dram_tensor("z_out", [Cm, LH], bf16, kind="ExternalOutput")
    bcr_out = nc.dram_tensor("bcr_out", [8, LH], bf16, kind="ExternalOutput")

    HP = HH + 2   # 66
    WP = W + 2    # 130

    with tile.TileContext(nc) as tc, \
         tc.tile_pool(name="w", bufs=1) as wp, \
         tc.tile_pool(name="d", bufs=1) as dp, \
         tc.tile_pool(name="ps", bufs=2, space="PSUM") as pp, \
         tc.tile_pool(name="ps8", bufs=2, space="PSUM") as pp8:
        tw1a = wp.tile([128, Cm], bf16)
        tw1b = wp.tile([128, Cm], bf16)
        nc.gpsimd.dma_start(out=tw1a, in_=wcv1[0:128, :])
        nc.gpsimd.dma_start(out=tw1b, in_=wcv1[128:256, :])
        twf = wp.tile([Cm, 9, Cm], bf16)
        nc.gpsimd.dma_start(out=twf, in_=wfold[:, :, :])
        twz = wp.tile([Cm, Cm], bf16)
        nc.gpsimd.dma_start(out=twz, in_=wz[:, :])
        twbc = wp.tile([Cm, 8], bf16)
        nc.gpsimd.dma_start(out=twbc, in_=wbc[:, :])
        tb1 = wp.tile([Cm, 1], f32)
        nc.sync.dma_start(out=tb1, in_=bcv1[:, :])
        tbc = wp.tile([Cm, 1], f32)
        nc.sync.dma_start(out=tbc, in_=bconv[:, :])
        tmask = wp.tile([Cm, 2], f32)
        nc.sync.dma_start(out=tmask, in_=hmask[:, :])

        txa = dp.tile([128, HP, W], bf16)
        txb = dp.tile([128, HP, W], bf16)
        for rb in range(0, HP, 11):
            nc.gpsimd.dma_start(out=txa[:, rb:rb + 11, :], in_=x_in[0:128, rb:rb + 11, :])
            nc.gpsimd.dma_start(out=txb[:, rb:rb + 11, :], in_=x_in[128:256, rb:rb + 11, :])

        th = dp.tile([Cm, HP, WP], bf16)
        nc.vector.memset(th[:, :, 0:1], 0.0)
        nc.vector.memset(th[:, :, WP - 1:WP], 0.0)

        # cv1 over 66 rows: 16 chunks of 4 rows + 1 chunk of 2 rows
        row_chunks = [(r0, 4) for r0 in range(0, 64, 4)] + [(64, 2)]
        for r0, nr in row_chunks:
            pt = pp.tile([Cm, 512], f32, tag="cv1")
            nn = nr * W
            nc.tensor.matmul(out=pt[:, :nn], lhsT=tw1a[:, :],
                             rhs=txa[:, r0:r0 + nr, :], start=True, stop=False)
            nc.tensor.matmul(out=pt[:, :nn], lhsT=tw1b[:, :],
                             rhs=txb[:, r0:r0 + nr, :], start=False, stop=True)
            nc.scalar.activation(out=th[:, r0:r0 + nr, 1:W + 1], in_=pt[:, :nn],
                                 func=AF.Identity, bias=tb1[:, :], scale=1.0)
        nc.vector.tensor_scalar_mul(out=th[:, 0, :], in0=th[:, 0, :],
                                    scalar1=tmask[:, 0:1])
        nc.vector.tensor_scalar_mul(out=th[:, HP - 1, :], in0=th[:, HP - 1, :],
                                    scalar1=tmask[:, 1:2])

        txc = dp.tile([Cm, HH, W], bf16)
        tz = dp.tile([Cm, HH, W], bf16)
        tbcr = dp.tile([8, LH], bf16)
        for r0 in range(0, HH, 4):
            pt = pp.tile([Cm, 512], f32, tag="fold")
            for t in range(9):
                dy, dx = t // 3 - 1, t % 3 - 1
                nc.tensor.matmul(
                    out=pt[:, :], lhsT=twf[:, t, :],
                    rhs=th[:, r0 + 1 + dy:r0 + 5 + dy, 1 + dx:W + 1 + dx],
                    start=(t == 0), stop=(t == 8))
            nc.scalar.activation(out=txc[:, r0:r0 + 4, :], in_=pt[:, :],
                                 func=AF.Silu, bias=tbc[:, :], scale=1.0)
            ptz = pp.tile([Cm, 512], f32, tag="z")
            nc.tensor.matmul(out=ptz[:, :], lhsT=twz[:, :],
                             rhs=th[:, r0 + 1:r0 + 5, 1:W + 1],
                             start=True, stop=True)
            nc.scalar.activation(out=tz[:, r0:r0 + 4, :], in_=ptz[:, :],
                                 func=AF.Silu, bias=0.0, scale=1.0)
            ptb = pp8.tile([8, 512], f32, tag="bc")
            nc.tensor.matmul(out=ptb[:, :], lhsT=twbc[:, :],
                             rhs=txc[:, r0:r0 + 4, :], start=True, stop=True)
            nc.vector.tensor_copy(out=tbcr[:, r0 * W:(r0 + 4) * W], in_=ptb[:, :])

            if r0 % 16 == 12:  # flush every 16 rows
                rs = r0 - 12
                nc.gpsimd.dma_start(out=xc_out[:, rs * W:(r0 + 4) * W],
                                    in_=txc[:, rs:r0 + 4, :])
                nc.gpsimd.dma_start(out=z_out[:, rs * W:(r0 + 4) * W],
                                    in_=tz[:, rs:r0 + 4, :])
        nc.gpsimd.dma_start(out=bcr_out[:, :], in_=tbcr[:, :])
    return nc


# ------------------------------------------------------------------- L2
def build_l2():
    nc = _new_nc()
    u_in = nc.dram_tensor("u_in", [Cm, L], bf16, kind="ExternalInput")
    wdt_f = nc.dram_tensor("wdt_f", [Cm, Cm], f32, kind="ExternalInput")
    wdt_r = nc.dram_tensor("wdt_r", [Cm, Cm], f32, kind="ExternalInput")
    dtb_f = nc.dram_tensor("dtb_f", [Cm, 1], f32, kind="ExternalInput")
    dtb_r = nc.dram_tensor("dtb_r", [Cm, 1], f32, kind="ExternalInput")
    a_f = nc.dram_tensor("a_f", [Cm, 1], f32, kind="ExternalInput")
    a_r = nc.dram_tensor("a_r", [Cm, 1], f32, kind="ExternalInput")
    brow_f = nc.dram_tensor("brow_f", [1, L], bf16, kind="ExternalInput")
    brow_r = nc.dram_tensor("brow_r", [1, L], bf16, kind="ExternalInput")
    crow_f = nc.dram_tensor("crow_f", [1, L], bf16, kind="ExternalInput")
    crow_r = nc.dram_tensor("crow_r", [1, L], bf16, kind="ExternalInput")
    ident = nc.dram_tensor("ident", [Cm, Cm], f32, kind="ExternalInput")
    diagd = nc.dram_tensor("diagd", [Cm, Cm], f32, kind="ExternalInput")
    m_out = nc.dram_tensor("m_out", [Cm, L], bf16, kind="ExternalOutput")

    def bc_ap(t, sl):  # DRAM row slice -> partition-replicated AP
        return bass.AP(tensor=t, offset=sl.start, ap=[[0, 128], [1, sl.stop - sl.start]])

    with tile.TileContext(nc) as tc, \
         tc.tile_pool(name="w", bufs=1) as wp, \
         tc.tile_pool(name="u", bufs=1) as up, \
         tc.tile_pool(name="full", bufs=1) as fp, \
         tc.tile_pool(name="ck", bufs=3) as cp, \
         tc.tile_pool(name="hk", bufs=3) as hp, \
         tc.tile_pool(name="bc", bufs=3) as bp, \
         tc.tile_pool(name="psd", bufs=1, space="PSUM") as psd, \
         tc.tile_pool(name="psm", bufs=2, space="PSUM") as psm:
        twf_ = wp.tile([Cm, Cm], bf16)
        twr_ = wp.tile([Cm, Cm], bf16)
        nc.gpsimd.dma_start(out=twr_, in_=wdt_r[:, :])
        nc.gpsimd.dma_start(out=twf_, in_=wdt_f[:, :])
        tbf = wp.tile([Cm, 1], f32)
        tbr = wp.tile([Cm, 1], f32)
        taf = wp.tile([Cm, 1], f32)
        tar = wp.tile([Cm, 1], f32)
        nc.sync.dma_start(out=tbf, in_=dtb_f[:, :])
        nc.sync.dma_start(out=tbr, in_=dtb_r[:, :])
        nc.sync.dma_start(out=taf, in_=a_f[:, :])
        nc.sync.dma_start(out=tar, in_=a_r[:, :])
        tid = wp.tile([Cm, Cm], bf16)
        tdg = wp.tile([Cm, Cm], bf16)
        nc.gpsimd.dma_start(out=tid, in_=ident[:, :])
        nc.gpsimd.dma_start(out=tdg, in_=diagd[:, :])

        tu = up.tile([Cm, L], bf16)
        for ci in range(NCH - 1, -1, -1):
            nc.sync.dma_start(out=tu[:, ci * CH:(ci + 1) * CH],
                              in_=u_in[:, ci * CH:(ci + 1) * CH])

        tmcr = fp.tile([Cm, L], bf16)   # h_r * C_r, natural position order

        def chunk_front(ci, tw, tb, ta, brow):
            """matmul + ACT chain + dt*u*B for chunk ci of one direction."""
            pt = psd.tile([Cm, CH], f32, tag="dtd")
            for j in range(CH // 512):
                nc.tensor.matmul(
                    out=pt[:, j * 512:(j + 1) * 512], lhsT=tw[:, :],
                    rhs=tu[:, ci * CH + j * 512: ci * CH + (j + 1) * 512],
                    start=True, stop=True)
            te1 = cp.tile([Cm, CH], bf16, tag="e1")
            nc.scalar.activation(out=te1, in_=pt[:, :], func=AF.Exp,
                                 bias=tb[:, :], scale=1.0)
            tdt = cp.tile([Cm, CH], bf16, tag="dt")
            nc.scalar.activation(out=tdt, in_=te1, func=AF.Ln, bias=1.0, scale=1.0)
            tav = cp.tile([Cm, CH], bf16, tag="av")
            nc.scalar.activation(out=tav, in_=tdt, func=AF.Exp,
                                 bias=0.0, scale=ta[:, :])
            sl = slice(ci * CH, (ci + 1) * CH)
            tbb = bp.tile([Cm, CH], bf16, tag="bbc")
            nc.gpsimd.dma_start(out=tbb, in_=bc_ap(brow, sl))
            tdtu = cp.tile([Cm, CH], bf16, tag="dtu")
            nc.vector.tensor_tensor(out=tdtu, in0=tdt, in1=tu[:, sl], op=MULT)
            tbt = cp.tile([Cm, CH], bf16, tag="bt")
            nc.vector.tensor_tensor(out=tbt, in0=tdtu, in1=tbb, op=MULT)
            return tav, tbt

        # ---- reverse direction: chunks descending, reversed-AP scan ----
        prev = None
        for ci in range(NCH - 1, -1, -1):
            tav, tbt = chunk_front(ci, twr_, tbr, tar, brow_r)
            thc = hp.tile([Cm, CH], bf16, tag="hr")
            nc.vector.tensor_tensor_scan(
                out=thc[:, ::-1], data0=tav[:, ::-1], data1=tbt[:, ::-1],
                initial=0.0 if prev is None else prev, op0=MULT, op1=ADD)
            prev = thc[:, 0:1]
            sl = slice(ci * CH, (ci + 1) * CH)
            tcc = bp.tile([Cm, CH], bf16, tag="cbc")
            nc.gpsimd.dma_start(out=tcc, in_=bc_ap(crow_r, sl))
            nc.vector.tensor_tensor(out=tmcr[:, sl], in0=thc, in1=tcc, op=MULT)

        # ---- forward direction + merge ----
        prev = None
        for ci in range(NCH):
            tav, tbt = chunk_front(ci, twf_, tbf, taf, brow_f)
            thc = hp.tile([Cm, CH], bf16, tag="hf")
            nc.vector.tensor_tensor_scan(
                out=thc, data0=tav, data1=tbt,
                initial=0.0 if prev is None else prev, op0=MULT, op1=ADD)
            prev = thc[:, CH - 1:CH]
            sl = slice(ci * CH, (ci + 1) * CH)
            tcc = bp.tile([Cm, CH], bf16, tag="cbc")
            nc.gpsimd.dma_start(out=tcc, in_=bc_ap(crow_f, sl))
            tmcf = hp.tile([Cm, CH], bf16, tag="mcf")
            nc.vector.tensor_tensor(out=tmcf, in0=thc, in1=tcc, op=MULT)
            tm = hp.tile([Cm, CH], bf16, tag="m")
            for q in range(CH // 1024):
                sq1 = slice(q * 1024, (q + 1) * 1024)
                pm = psm.tile([Cm, 1024], f32, tag="mp")
                for v in range(2):
                    sq = slice(v * 512, (v + 1) * 512)
                    sqg = slice(ci * CH + q * 1024 + v * 512,
                                ci * CH + q * 1024 + (v + 1) * 512)
                    sqf = slice(q * 1024 + v * 512, q * 1024 + (v + 1) * 512)
                    nc.tensor.matmul(out=pm[:, sq], lhsT=tid[:, :], rhs=tmcf[:, sqf],
                                     start=True, stop=False)
                    nc.tensor.matmul(out=pm[:, sq], lhsT=tid[:, :], rhs=tmcr[:, sqg],
                                     start=False, stop=False)
                    nc.tensor.matmul(out=pm[:, sq], lhsT=tdg[:, :], rhs=tu[:, sqg],
                                     start=False, stop=True)
                nc.scalar.activation(out=tm[:, sq1], in_=pm[:, :],
                                     func=AF.Identity, bias=0.0, scale=1.0)
            nc.gpsimd.dma_start(out=m_out[:, sl], in_=tm)
    return nc


# ------------------------------------------------------------------- L3
def build_l3():
    nc = _new_nc()
    m02 = nc.dram_tensor("m02", [Cm, LH], bf16, kind="ExternalInput")
    m13 = nc.dram_tensor("m13", [Cm, LH], bf16, kind="ExternalInput")
    z_in = nc.dram_tensor("z_in", [Cm, LH], bf16, kind="ExternalInput")
    lng = nc.dram_tensor("lng", [1, Cm], f32, kind="ExternalInput")
    lnb = nc.dram_tensor("lnb", [1, Cm], f32, kind="ExternalInput")
    wfin = nc.dram_tensor("wfin", [Cm, C2], f32, kind="ExternalInput")
    bfin = nc.dram_tensor("bfin", [128, 2], f32, kind="ExternalInput")
    ones128 = nc.dram_tensor("ones128", [Cm, 1], f32, kind="ExternalInput")
    onesrow = nc.dram_tensor("onesrow", [1, LH], f32, kind="ExternalInput")
    d_out = nc.dram_tensor("d_out", [C2, LH], f32, kind="ExternalOutput")

    NC3 = LH // CH  # 4
    QL = LH // 128  # 64

    with tile.TileContext(nc) as tc, \
         tc.tile_pool(name="w", bufs=1) as wp, \
         tc.tile_pool(name="d", bufs=1) as dp, \
         tc.tile_pool(name="c", bufs=3) as cp, \
         tc.tile_pool(name="st", bufs=1) as sp, \
         tc.tile_pool(name="ps1", bufs=2, space="PSUM") as ps1, \
         tc.tile_pool(name="ps2", bufs=2, space="PSUM") as ps2:
        tg = wp.tile([1, Cm], bf16)
        tb = wp.tile([1, Cm], bf16)
        nc.gpsimd.dma_start(out=tg, in_=lng[:, :])
        nc.gpsimd.dma_start(out=tb, in_=lnb[:, :])
        twa = wp.tile([Cm, 128], bf16)
        twb = wp.tile([Cm, 128], bf16)
        nc.gpsimd.dma_start(out=twa, in_=wfin[:, 0:128])
        nc.gpsimd.dma_start(out=twb, in_=wfin[:, 128:256])
        tbf = wp.tile([128, 2], f32)
        nc.sync.dma_start(out=tbf, in_=bfin[:, :])
        tone = wp.tile([Cm, 1], bf16)
        nc.gpsimd.dma_start(out=tone, in_=ones128[:, :])
        tonesrow = wp.tile([1, LH], bf16)
        nc.gpsimd.dma_start(out=tonesrow, in_=onesrow[:, :])
        teps = wp.tile([128, 1], f32)
        nc.vector.memset(teps, 1e-5)

        tm0 = dp.tile([Cm, LH], bf16)
        tm1 = dp.tile([Cm, LH], bf16)
        tz = dp.tile([Cm, LH], bf16)
        for hh in range(2):
            s = slice(hh * LH // 2, (hh + 1) * LH // 2)
            nc.sync.dma_start(out=tm0[:, s], in_=m02[:, s])
            nc.sync.dma_start(out=tm1[:, s], in_=m13[:, s])
            nc.sync.dma_start(out=tz[:, s], in_=z_in[:, s])

        ty = dp.tile([Cm, LH], bf16)
        tmu = sp.tile([1, LH], bf16, tag="mu")
        tss = sp.tile([1, LH], bf16, tag="ss")
        for ci in range(NC3):
            sl = slice(ci * CH, (ci + 1) * CH)
            nc.vector.tensor_tensor(out=ty[:, sl], in0=tm0[:, sl],
                                    in1=tm1[:, sl], op=ADD)
            tsq = cp.tile([Cm, CH], bf16, tag="sq")
            nc.vector.tensor_tensor(out=tsq, in0=ty[:, sl], in1=ty[:, sl], op=MULT)
            for j in range(CH // 512):
                s2 = slice(j * 512, (j + 1) * 512)
                s2g = slice(ci * CH + j * 512, ci * CH + (j + 1) * 512)
                pmu = ps2.tile([1, 512], f32, tag="da")
                pss = ps2.tile([1, 512], f32, tag="db")
                nc.tensor.matmul(out=pmu[:, :], lhsT=tone[:, :],
                                 rhs=ty[:, s2g], start=True, stop=True)
                nc.tensor.matmul(out=pss[:, :], lhsT=tone[:, :],
                                 rhs=tsq[:, s2], start=True, stop=True)
                nc.scalar.activation(out=tmu[:, s2g], in_=pmu[:, :],
                                     func=AF.Identity, bias=0.0, scale=1.0)
                nc.scalar.activation(out=tss[:, s2g], in_=pss[:, :],
                                     func=AF.Identity, bias=0.0, scale=1.0)

        tmu2 = sp.tile([128, QL], f32, tag="r1")
        tss2 = sp.tile([128, QL], f32, tag="r2")
        nc.gpsimd.dma_start(out=tmu2[:, :], in_=tmu[0:1, :])
        nc.gpsimd.dma_start(out=tss2[:, :], in_=tss[0:1, :])
        tvar = sp.tile([128, QL], f32, tag="r3")
        nc.vector.tensor_tensor(out=tvar, in0=tmu2, in1=tmu2, op=MULT)
        nc.vector.tensor_tensor(out=tvar, in0=tss2, in1=tvar, op=SUB)
        tlnv = sp.tile([128, QL], f32, tag="r4")
        nc.scalar.activation(out=tlnv, in_=tvar, func=AF.Ln,
                             bias=teps[:, :], scale=1.0)
        trst = sp.tile([128, QL], f32, tag="r5")
        nc.scalar.activation(out=trst, in_=tlnv, func=AF.Exp,
                             bias=0.0, scale=-0.5)
        tmr = sp.tile([128, QL], f32, tag="r6")
        nc.vector.tensor_tensor(out=tmr, in0=tmu2, in1=trst, op=MULT)
        nc.vector.tensor_scalar_mul(out=tmr, in0=tmr, scalar1=-1.0)
        trow_r = sp.tile([1, LH], bf16, tag="r7")
        trow_m = sp.tile([1, LH], bf16, tag="r8")
        nc.gpsimd.dma_start(out=trow_r[0:1, :], in_=trst[:, :])
        nc.gpsimd.dma_start(out=trow_m[0:1, :], in_=tmr[:, :])

        for ci in range(NC3):
            for j in range(CH // 512):
                s2g = slice(ci * CH + j * 512, ci * CH + (j + 1) * 512)
                pR = ps1.tile([Cm, 512], f32, tag="R")
                pS = ps1.tile([Cm, 512], f32, tag="S")
                nc.tensor.matmul(out=pR[:, :], lhsT=tg[:, :],
                                 rhs=trow_r[:, s2g], start=True, stop=True)
                nc.tensor.matmul(out=pS[:, :], lhsT=tg[:, :],
                                 rhs=trow_m[:, s2g], start=True, stop=False)
                nc.tensor.matmul(out=pS[:, :], lhsT=tb[:, :],
                                 rhs=tonesrow[:, s2g], start=False, stop=True)
                tt = cp.tile([Cm, 512], bf16, tag="t")
                nc.vector.tensor_tensor(out=tt, in0=ty[:, s2g], in1=pR[:, :], op=MULT)
                nc.vector.tensor_tensor(out=tt, in0=tt, in1=pS[:, :], op=ADD)
                nc.vector.tensor_tensor(out=tt, in0=tt, in1=tz[:, s2g], op=MULT)
                pda = ps2.tile([128, 512], f32, tag="da")
                pdb = ps2.tile([128, 512], f32, tag="db")
                nc.tensor.matmul(out=pda[:, :], lhsT=twa[:, :], rhs=tt[:, :],
                                 start=True, stop=True)
                nc.tensor.matmul(out=pdb[:, :], lhsT=twb[:, :], rhs=tt[:, :],
                                 start=True, stop=True)
                tda = cp.tile([128, 512], f32, tag="oa")
                tdb = cp.tile([128, 512], f32, tag="ob")
                nc.scalar.activation(out=tda, in_=pda[:, :], func=AF.Identity,
                                     bias=tbf[:, 0:1], scale=1.0)
                nc.scalar.activation(out=tdb, in_=pdb[:, :], func=AF.Identity,
                                     bias=tbf[:, 1:2], scale=1.0)
                nc.sync.dma_start(out=d_out[0:128, s2g], in_=tda)
                nc.sync.dma_start(out=d_out[128:256, s2g], in_=tdb)
    return nc


# ------------------------------------------------------------------- host
def _get_ncs():
    if "ncs" not in _CACHE:
        nc1, nc2, nc3 = build_l1(), build_l2(), build_l3()
        for n in (nc1, nc2, nc3):
            _split_multiwaits(n)
        _CACHE["ncs"] = (nc1, nc2, nc3)
    return _CACHE["ncs"]


def kernel(x, cv1_w, cv1_b, scale_w, in_proj_w, conv_w, conv_b, x_proj_w,
           dt_w, dt_b, A_logs, Ds, ln_g, ln_b, out_proj_w, cv2_w, cv2_b):
    f = np.float32
    x = np.asarray(x, f)
    cv1_w = np.asarray(cv1_w, f); cv1_b = np.asarray(cv1_b, f)
    in_proj_w = np.asarray(in_proj_w, f)
    conv_w = np.asarray(conv_w, f); conv_b = np.asarray(conv_b, f)
    x_proj_w = np.asarray(x_proj_w, f)
    dt_w = np.asarray(dt_w, f); dt_b = np.asarray(dt_b, f)
    A_logs = np.asarray(A_logs, f); Ds = np.asarray(Ds, f)
    ln_g = np.asarray(ln_g, f); ln_b = np.asarray(ln_b, f)
    out_proj_w = np.asarray(out_proj_w, f)
    cv2_w = np.asarray(cv2_w, f); cv2_b = np.asarray(cv2_b, f)
    scale_v = np.asarray(scale_w, f).reshape(Cm)

    Wip_x, Wip_z = in_proj_w[:Cm], in_proj_w[Cm:]
    dwk = conv_w[:, 0]
    A = -np.exp(A_logs).reshape(K, Cm)
    Dk = Ds.reshape(K, Cm)
    W_dtk = np.einsum('kdr,krc->kdc', dt_w, x_proj_w[:, :R])
    WB, WC = x_proj_w[:, R], x_proj_w[:, R + 1]
    W_final = cv2_w @ (scale_v[:, None] * out_proj_w)

    # fold lhsT: (tap, k=h-chan, m=out-chan) -> host layout (k, tap, m)
    Wfold = np.einsum('cyx,cd->yxdc', dwk, Wip_x)      # (3,3, in, out)
    wfold_rm = np.ascontiguousarray(
        Wfold.reshape(9, Cm, Cm).transpose(1, 0, 2))   # row-major cores
    wbc_l = np.stack([WB[0], WC[0], WB[2], WC[2],
                      WB[1], WC[1], WB[3], WC[3]], axis=1)

    nc1, nc2, nc3 = _get_ncs()

    # ---------------- L1 ----------------
    l1_maps = []
    for core in range(8):
        b, half = core // 2, core % 2
        r0 = half * HH
        xs = np.zeros((C1, HH + 2, W), np.float32)
        lo, hi = r0 - 1, r0 + HH + 1
        slo, shi = max(lo, 0), min(hi, H)
        xs[:, slo - lo: shi - lo, :] = x[b, :, slo:shi, :]
        mask = np.ones((Cm, 2), np.float32)
        mask[:, 0] = 0.0 if half == 0 else 1.0
        mask[:, 1] = 1.0 if half == 0 else 0.0
        l1_maps.append({
            "x_in": xs,
            "wcv1": np.ascontiguousarray(cv1_w.T),
            "bcv1": cv1_b.reshape(Cm, 1),
            "wfold": wfold_rm,
            "bconv": conv_b.reshape(Cm, 1),
            "wz": np.ascontiguousarray(Wip_z.T),
            "wbc": np.ascontiguousarray(wbc_l),
            "hmask": mask,
        })
    r1 = _run(nc1, l1_maps, "L1")

    xc = np.zeros((B, Cm, L), NBF)
    zf = np.zeros((B, Cm, L), NBF)
    rows = np.zeros((B, 8, L), NBF)
    for core in range(8):
        b, half = core // 2, core % 2
        sl = slice(half * LH, (half + 1) * LH)
        xc[b][:, sl] = r1[core]["xc_out"]
        zf[b][:, sl] = r1[core]["z_out"]
        rows[b][:, sl] = r1[core]["bcr_out"]

    # ---------------- L2 ----------------
    def t_spatial(a):
        return np.ascontiguousarray(
            a.reshape(*a.shape[:-1], H, W).swapaxes(-1, -2).reshape(*a.shape[:-1], L))

    ident = np.eye(Cm, dtype=np.float32)
    l2_maps = []
    for core in range(8):
        b, g = core // 2, core % 2
        if g == 0:
            u = xc[b]
            kf, kr = 0, 2
            br_f, cr_f = rows[b][0], rows[b][1]
            br_r, cr_r = rows[b][2], rows[b][3]
        else:
            u = t_spatial(xc[b])
            kf, kr = 1, 3
            br_f, cr_f = t_spatial(rows[b][4]), t_spatial(rows[b][5])
            br_r, cr_r = t_spatial(rows[b][6]), t_spatial(rows[b][7])
        dsum_v = (Dk[kf] + Dk[kr]).astype(np.float32)
        l2_maps.append({
            "u_in": np.ascontiguousarray(u),
            "wdt_f": np.ascontiguousarray(W_dtk[kf].T),
            "wdt_r": np.ascontiguousarray(W_dtk[kr].T),
            "dtb_f": dt_b[kf].reshape(Cm, 1), "dtb_r": dt_b[kr].reshape(Cm, 1),
            "a_f": A[kf].reshape(Cm, 1).astype(f), "a_r": A[kr].reshape(Cm, 1).astype(f),
            "brow_f": br_f.reshape(1, L), "brow_r": br_r.reshape(1, L),
            "crow_f": cr_f.reshape(1, L), "crow_r": cr_r.reshape(1, L),
            "ident": ident, "diagd": np.diag(dsum_v).astype(np.float32),
        })
    r2 = _run(nc2, l2_maps, "L2")

    # ---------------- L3 ----------------
    l3_maps = []
    for b in range(B):
        m02 = r2[2 * b]["m_out"]
        m13t = t_spatial(r2[2 * b + 1]["m_out"])
        for half in range(2):
            sl = slice(half * LH, (half + 1) * LH)
            l3_maps.append({
                "m02": np.ascontiguousarray(m02[:, sl]),
                "m13": np.ascontiguousarray(m13t[:, sl]),
                "z_in": np.ascontiguousarray(zf[b][:, sl]),
                "lng": ln_g.reshape(1, Cm),
                "lnb": ln_b.reshape(1, Cm),
                "wfin": np.ascontiguousarray(W_final.T),
                "bfin": np.ascontiguousarray(cv2_b.reshape(2, 128).T),
                "ones128": np.full((Cm, 1), 1.0 / Cm, np.float32),
                "onesrow": np.ones((1, LH), np.float32),
            })
    r3 = _run(nc3, l3_maps, "L3")

    out = np.empty((B, C2, H, W), np.float32)
    for core in range(8):
        b, half = core // 2, core % 2
        sl = slice(half * LH, (half + 1) * LH)
        out[b].reshape(C2, L)[:, sl] = r3[core]["d_out"]
    out += x
    return out



# revision 3
# speedup vs baseline: 1.1730x; 1.1730x over previous
"""BottleneckMamba Trainium2 kernel (self-contained).

out = x + cv2( scale * out_proj( LN(cross-merge(4-dir selective scan(N=1))) * z ) )

3 SPMD launches on 8 NeuronCores:
  L1 (core=(b, image-half)): cv1 -> h; depthwise3x3*in_proj folded into 9
     matmuls -> silu -> xc ; z = silu(Wz@h) ; B/C projection rows.
  L2 (core=(b, dir-group)): g=0 dirs {0,2} on xc row-major; g=1 dirs {1,3} on
     host-transposed xc (identical program). Per dir: dtd matmul ->
     exp/ln1p/exp on ACT (softplus+decay, one table set) -> dt*u*Bbc ->
     tensor_tensor_scan (reversed-AP traversal for the backward dir) ->
     h*Cbc ; PE merges the pair + D*u -> m.
  L3 (core=(b, half)): y = m02 + m13^T (host transposes m13), LayerNorm via
     matmul stats + rank-1 broadcast matmuls, *z, fused
     (cv2 @ diag(scale) @ out_proj) matmul + bias -> delta.
Host: shards/reassembles, transposes between launches, adds residual x.
"""
import os
import sys

sys.path.insert(0, '/opt/trn_rl_repo')

import numpy as np
import ml_dtypes

import concourse.bass as bass
import concourse.tile as tile
import concourse.mybir as mybir
from concourse.bass_utils import run_bass_kernel_spmd

bf16 = mybir.dt.bfloat16
f32 = mybir.dt.float32
MULT, ADD = mybir.AluOpType.mult, mybir.AluOpType.add
SUB = mybir.AluOpType.subtract
AF = mybir.ActivationFunctionType
NBF = ml_dtypes.bfloat16

B, C1, C2, H, W = 4, 256, 256, 128, 128
Cm, K, R = 128, 4, 8
L = H * W          # 16384
HH = H // 2        # 64 rows per half
LH = HH * W        # 8192
CH = 2048          # L2/L3 chunk
NCH = L // CH      # 8

EXEC_TIMES = {}    # launch -> exec ns (MAMBA_TRACE=1)
TRACES = {}        # launch -> (insts, trace_path) (MAMBA_TRACE=1)
_CACHE = {}


def _split_multiwaits(nc):
    """walrus here accepts ONE sync-wait per instruction; hoist extras into
    single-wait same-engine NOPs inserted before the instruction."""
    for f in nc.m.functions:
        for bb in f.blocks:
            il = bb.instructions
            i = 0
            while i < len(il):
                ins = il[i]
                si = getattr(ins, "sync_info", None)
                if si is not None and len(si.on_wait) > 1:
                    waits = list(si.on_wait)
                    ins.sync_info = mybir.SyncInfo(
                        on_wait=[waits[-1]], on_update=list(si.on_update))
                    for w in waits[:-1]:
                        nop = mybir.InstNoOp(
                            name=nc.get_next_instruction_name(), ins=[], outs=[])
                        nop.engine = ins.engine
                        nop.sync_info = mybir.SyncInfo(on_wait=[w], on_update=[])
                        nc.register_instruction(nop, overwrite=True)
                        il.insert(i, nop)
                        i += 1
                i += 1


def _new_nc():
    return bass.Bass("TRN2", target_bir_lowering=False, debug=False,
                     enable_asserts=True, num_devices=8)


def _run(nc, in_maps, name):
    trace = os.environ.get("MAMBA_TRACE", "0") == "1"
    res = run_bass_kernel_spmd(nc, in_maps, core_ids=list(range(8)), trace=trace)
    if trace:
        EXEC_TIMES[name] = res.exec_time_ns
        TRACES[name] = res.instructions_and_trace
    return res.results


# ------------------------------------------------------------------- L1
def build_l1():
    nc = _new_nc()
    x_in = nc.dram_tensor("x_in", [C1, HH + 2, W], f32, kind="ExternalInput")
    wcv1 = nc.dram_tensor("wcv1", [C1, Cm], f32, kind="ExternalInput")       # lhsT
    bcv1 = nc.dram_tensor("bcv1", [Cm, 1], f32, kind="ExternalInput")
    wfold = nc.dram_tensor("wfold", [Cm, 9, Cm], f32, kind="ExternalInput")  # (k, tap, m)
    bconv = nc.dram_tensor("bconv", [Cm, 1], f32, kind="ExternalInput")
    wz = nc.dram_tensor("wz", [Cm, Cm], f32, kind="ExternalInput")           # lhsT
    wbc = nc.dram_tensor("wbc", [Cm, 8], f32, kind="ExternalInput")          # lhsT
    hmask = nc.dram_tensor("hmask", [Cm, 2], f32, kind="ExternalInput")
    xc_out = nc.dram_tensor("xc_out", [Cm, LH], bf16, kind="ExternalOutput")
    z_out = nc.dram_tensor("z_out", [Cm, LH], bf16, kind="ExternalOutput")
    bcr_out = nc.dram_tensor("bcr_out", [8, LH], bf16, kind="ExternalOutput")

    HP = HH + 2   # 66
    WP = W + 2    # 130

    with tile.TileContext(nc) as tc, \
         tc.tile_pool(name="w", bufs=1) as wp, \
         tc.tile_pool(name="d", bufs=1) as dp, \
         tc.tile_pool(name="ps", bufs=2, space="PSUM") as pp, \
         tc.tile_pool(name="ps8", bufs=2, space="PSUM") as pp8:
        tw1a = wp.tile([128, Cm], bf16)
        tw1b = wp.tile([128, Cm], bf16)
        nc.gpsimd.dma_start(out=tw1a, in_=wcv1[0:128, :])
        nc.gpsimd.dma_start(out=tw1b, in_=wcv1[128:256, :])
        twf = wp.tile([Cm, 9, Cm], bf16)
        nc.gpsimd.dma_start(out=twf, in_=wfold[:, :, :])
        twz = wp.tile([Cm, Cm], bf16)
        nc.gpsimd.dma_start(out=twz, in_=wz[:, :])
        twbc = wp.tile([Cm, 8], bf16)
        nc.gpsimd.dma_start(out=twbc, in_=wbc[:, :])
        tb1 = wp.tile([Cm, 1], f32)
        nc.sync.dma_start(out=tb1, in_=bcv1[:, :])
        tbc = wp.tile([Cm, 1], f32)
        nc.sync.dma_start(out=tbc, in_=bconv[:, :])
        tmask = wp.tile([Cm, 2], f32)
        nc.sync.dma_start(out=tmask, in_=hmask[:, :])

        txa = dp.tile([128, HP, W], bf16)
        txb = dp.tile([128, HP, W], bf16)
        for rb in range(0, HP, 11):
            nc.gpsimd.dma_start(out=txa[:, rb:rb + 11, :], in_=x_in[0:128, rb:rb + 11, :])
            nc.gpsimd.dma_start(out=txb[:, rb:rb + 11, :], in_=x_in[128:256, rb:rb + 11, :])

        th = dp.tile([Cm, HP, WP], bf16)
        nc.vector.memset(th[:, :, 0:1], 0.0)
        nc.vector.memset(th[:, :, WP - 1:WP], 0.0)

        # cv1 over 66 rows: 16 chunks of 4 rows + 1 chunk of 2 rows
        row_chunks = [(r0, 4) for r0 in range(0, 64, 4)] + [(64, 2)]
        for r0, nr in row_chunks:
            pt = pp.tile([Cm, 512], f32, tag="cv1")
            nn = nr * W
            nc.tensor.matmul(out=pt[:, :nn], lhsT=tw1a[:, :],
                             rhs=txa[:, r0:r0 + nr, :], start=True, stop=False)
            nc.tensor.matmul(out=pt[:, :nn], lhsT=tw1b[:, :],
                             rhs=txb[:, r0:r0 + nr, :], start=False, stop=True)
            nc.scalar.activation(out=th[:, r0:r0 + nr, 1:W + 1], in_=pt[:, :nn],
                                 func=AF.Identity, bias=tb1[:, :], scale=1.0)
        nc.vector.tensor_scalar_mul(out=th[:, 0, :], in0=th[:, 0, :],
                                    scalar1=tmask[:, 0:1])
        nc.vector.tensor_scalar_mul(out=th[:, HP - 1, :], in0=th[:, HP - 1, :],
                                    scalar1=tmask[:, 1:2])

        txc = dp.tile([Cm, HH, W], bf16)
        tz = dp.tile([Cm, HH, W], bf16)
        tbcr = dp.tile([8, LH], bf16)
        for r0 in range(0, HH, 4):
            pt = pp.tile([Cm, 512], f32, tag="fold")
            for t in range(9):
                dy, dx = t // 3 - 1, t % 3 - 1
                nc.tensor.matmul(
                    out=pt[:, :], lhsT=twf[:, t, :],
                    rhs=th[:, r0 + 1 + dy:r0 + 5 + dy, 1 + dx:W + 1 + dx],
                    start=(t == 0), stop=(t == 8))
            nc.scalar.activation(out=txc[:, r0:r0 + 4, :], in_=pt[:, :],
                                 func=AF.Silu, bias=tbc[:, :], scale=1.0)
            ptz = pp.tile([Cm, 512], f32, tag="z")
            nc.tensor.matmul(out=ptz[:, :], lhsT=twz[:, :],
                             rhs=th[:, r0 + 1:r0 + 5, 1:W + 1],
                             start=True, stop=True)
            nc.scalar.activation(out=tz[:, r0:r0 + 4, :], in_=ptz[:, :],
                                 func=AF.Silu, bias=0.0, scale=1.0)
            ptb = pp8.tile([8, 512], f32, tag="bc")
            nc.tensor.matmul(out=ptb[:, :], lhsT=twbc[:, :],
                             rhs=txc[:, r0:r0 + 4, :], start=True, stop=True)
            nc.vector.tensor_copy(out=tbcr[:, r0 * W:(r0 + 4) * W], in_=ptb[:, :])

            if r0 % 16 == 12:  # flush every 16 rows
                rs = r0 - 12
                nc.gpsimd.dma_start(out=xc_out[:, rs * W:(r0 + 4) * W],
                                    in_=txc[:, rs:r0 + 4, :])
                nc.gpsimd.dma_start(out=z_out[:, rs * W:(r0 + 4) * W],
                                    in_=tz[:, rs:r0 + 4, :])
        nc.gpsimd.dma_start(out=bcr_out[:, :], in_=tbcr[:, :])
    return nc


# ------------------------------------------------------------------- L2
def build_l2():
    nc = _new_nc()
    u_in = nc.dram_tensor("u_in", [Cm, L], bf16, kind="ExternalInput")
    wdt_f = nc.dram_tensor("wdt_f", [Cm, Cm], f32, kind="ExternalInput")
    wdt_r = nc.dram_tensor("wdt_r", [Cm, Cm], f32, kind="ExternalInput")
    dtb_f = nc.dram_tensor("dtb_f", [Cm, 1], f32, kind="ExternalInput")
    dtb_r = nc.dram_tensor("dtb_r", [Cm, 1], f32, kind="ExternalInput")
    a_f = nc.dram_tensor("a_f", [Cm, 1], f32, kind="ExternalInput")
    a_r = nc.dram_tensor("a_r", [Cm, 1], f32, kind="ExternalInput")
    brow_f = nc.dram_tensor("brow_f", [1, L], bf16, kind="ExternalInput")
    brow_r = nc.dram_tensor("brow_r", [1, L], bf16, kind="ExternalInput")
    crow_f = nc.dram_tensor("crow_f", [1, L], bf16, kind="ExternalInput")
    crow_r = nc.dram_tensor("crow_r", [1, L], bf16, kind="ExternalInput")
    ident = nc.dram_tensor("ident", [Cm, Cm], f32, kind="ExternalInput")
    diagd = nc.dram_tensor("diagd", [Cm, Cm], f32, kind="ExternalInput")
    m_out = nc.dram_tensor("m_out", [Cm, L], bf16, kind="ExternalOutput")

    def bc_ap(t, sl):  # DRAM row slice -> partition-replicated AP
        return bass.AP(tensor=t, offset=sl.start, ap=[[0, 128], [1, sl.stop - sl.start]])

    with tile.TileContext(nc) as tc, \
         tc.tile_pool(name="w", bufs=1) as wp, \
         tc.tile_pool(name="u", bufs=1) as up, \
         tc.tile_pool(name="full", bufs=1) as fp, \
         tc.tile_pool(name="ck", bufs=3) as cp, \
         tc.tile_pool(name="hk", bufs=3) as hp, \
         tc.tile_pool(name="bc", bufs=3) as bp, \
         tc.tile_pool(name="psd", bufs=1, space="PSUM") as psd, \
         tc.tile_pool(name="psm", bufs=2, space="PSUM") as psm:
        twf_ = wp.tile([Cm, Cm], bf16)
        twr_ = wp.tile([Cm, Cm], bf16)
        nc.gpsimd.dma_start(out=twr_, in_=wdt_r[:, :])
        nc.gpsimd.dma_start(out=twf_, in_=wdt_f[:, :])
        tbf = wp.tile([Cm, 1], f32)
        tbr = wp.tile([Cm, 1], f32)
        taf = wp.tile([Cm, 1], f32)
        tar = wp.tile([Cm, 1], f32)
        nc.sync.dma_start(out=tbf, in_=dtb_f[:, :])
        nc.sync.dma_start(out=tbr, in_=dtb_r[:, :])
        nc.sync.dma_start(out=taf, in_=a_f[:, :])
        nc.sync.dma_start(out=tar, in_=a_r[:, :])
        tid = wp.tile([Cm, Cm], bf16)
        tdg = wp.tile([Cm, Cm], bf16)
        nc.gpsimd.dma_start(out=tid, in_=ident[:, :])
        nc.gpsimd.dma_start(out=tdg, in_=diagd[:, :])

        tu = up.tile([Cm, L], bf16)
        for ci in range(NCH - 1, -1, -1):
            nc.sync.dma_start(out=tu[:, ci * CH:(ci + 1) * CH],
                              in_=u_in[:, ci * CH:(ci + 1) * CH])

        tmcr = fp.tile([Cm, L], bf16)   # h_r * C_r, natural position order

        def chunk_front(ci, tw, tb, ta, brow):
            """matmul + ACT chain + dt*u*B for chunk ci of one direction."""
            pt = psd.tile([Cm, CH], f32, tag="dtd")
            for j in range(CH // 512):
                nc.tensor.matmul(
                    out=pt[:, j * 512:(j + 1) * 512], lhsT=tw[:, :],
                    rhs=tu[:, ci * CH + j * 512: ci * CH + (j + 1) * 512],
                    start=True, stop=True)
            te1 = cp.tile([Cm, CH], bf16, tag="e1")
            nc.scalar.activation(out=te1, in_=pt[:, :], func=AF.Exp,
                                 bias=tb[:, :], scale=1.0)
            tdt = cp.tile([Cm, CH], bf16, tag="dt")
            nc.scalar.activation(out=tdt, in_=te1, func=AF.Ln, bias=1.0, scale=1.0)
            tav = cp.tile([Cm, CH], bf16, tag="av")
            nc.scalar.activation(out=tav, in_=tdt, func=AF.Exp,
                                 bias=0.0, scale=ta[:, :])
            sl = slice(ci * CH, (ci + 1) * CH)
            tbb = bp.tile([Cm, CH], bf16, tag="bbc")
            nc.gpsimd.dma_start(out=tbb, in_=bc_ap(brow, sl))
            tdtu = cp.tile([Cm, CH], bf16, tag="dtu")
            nc.vector.tensor_tensor(out=tdtu, in0=tdt, in1=tu[:, sl], op=MULT)
            tbt = cp.tile([Cm, CH], bf16, tag="bt")
            nc.vector.tensor_tensor(out=tbt, in0=tdtu, in1=tbb, op=MULT)
            return tav, tbt

        # ---- reverse direction: chunks descending, reversed-AP scan ----
        prev = None
        for ci in range(NCH - 1, -1, -1):
            tav, tbt = chunk_front(ci, twr_, tbr, tar, brow_r)
            thc = hp.tile([Cm, CH], bf16, tag="hr")
            nc.vector.tensor_tensor_scan(
                out=thc[:, ::-1], data0=tav[:, ::-1], data1=tbt[:, ::-1],
                initial=0.0 if prev is None else prev, op0=MULT, op1=ADD)
            prev = thc[:, 0:1]
            sl = slice(ci * CH, (ci + 1) * CH)
            tcc = bp.tile([Cm, CH], bf16, tag="cbc")
            nc.gpsimd.dma_start(out=tcc, in_=bc_ap(crow_r, sl))
            nc.vector.tensor_tensor(out=tmcr[:, sl], in0=thc, in1=tcc, op=MULT)

        # ---- forward direction + merge ----
        prev = None
        for ci in range(NCH):
            tav, tbt = chunk_front(ci, twf_, tbf, taf, brow_f)
            thc = hp.tile([Cm, CH], bf16, tag="hf")
            nc.vector.tensor_tensor_scan(
                out=thc, data0=tav, data1=tbt,
                initial=0.0 if prev is None else prev, op0=MULT, op1=ADD)
            prev = thc[:, CH - 1:CH]
            sl = slice(ci * CH, (ci + 1) * CH)
            tcc = bp.tile([Cm, CH], bf16, tag="cbc")
            nc.gpsimd.dma_start(out=tcc, in_=bc_ap(crow_f, sl))
            tmcf = hp.tile([Cm, CH], bf16, tag="mcf")
            nc.vector.tensor_tensor(out=tmcf, in0=thc, in1=tcc, op=MULT)
            tm = hp.tile([Cm, CH], bf16, tag="m")
            for q in range(CH // 1024):
                sq1 = slice(q * 1024, (q + 1) * 1024)
                pm = psm.tile([Cm, 1024], f32, tag="mp")
                for v in range(2):
                    sq = slice(v * 512, (v + 1) * 512)
                    sqg = slice(ci * CH + q * 1024 + v * 512,
                                ci * CH + q * 1024 + (v + 1) * 512)
                    sqf = slice(q * 1024 + v * 512, q * 1024 + (v + 1) * 512)
                    nc.tensor.matmul(out=pm[:, sq], lhsT=tid[:, :], rhs=tmcf[:, sqf],
                                     start=True, stop=False)
                    nc.tensor.matmul(out=pm[:, sq], lhsT=tid[:, :], rhs=tmcr[:, sqg],
                                     start=False, stop=False)
                    nc.tensor.matmul(out=pm[:, sq], lhsT=tdg[:, :], rhs=tu[:, sqg],
                                     start=False, stop=True)
                nc.scalar.activation(out=tm[:, sq1], in_=pm[:, :],
                                     func=AF.Identity, bias=0.0, scale=1.0)
            nc.gpsimd.dma_start(out=m_out[:, sl], in_=tm)
    return nc


# ------------------------------------------------------------------- L3
def build_l3():
    nc = _new_nc()
    m02 = nc.dram_tensor("m02", [Cm, LH], bf16, kind="ExternalInput")
    m13 = nc.dram_tensor("m13", [Cm, LH], bf16, kind="ExternalInput")
    z_in = nc.dram_tensor("z_in", [Cm, LH], bf16, kind="ExternalInput")
    lng = nc.dram_tensor("lng", [1, Cm], f32, kind="ExternalInput")
    lnb = nc.dram_tensor("lnb", [1, Cm], f32, kind="ExternalInput")
    wfin = nc.dram_tensor("wfin", [Cm, C2], f32, kind="ExternalInput")
    bfin = nc.dram_tensor("bfin", [128, 2], f32, kind="ExternalInput")
    ones128 = nc.dram_tensor("ones128", [Cm, 1], f32, kind="ExternalInput")
    onesrow = nc.dram_tensor("onesrow", [1, LH], f32, kind="ExternalInput")
    d_out = nc.dram_tensor("d_out", [C2, LH], f32, kind="ExternalOutput")

    NC3 = LH // CH  # 4
    QL = LH // 128  # 64

    with tile.TileContext(nc) as tc, \
         tc.tile_pool(name="w", bufs=1) as wp, \
         tc.tile_pool(name="d", bufs=1) as dp, \
         tc.tile_pool(name="c", bufs=3) as cp, \
         tc.tile_pool(name="st", bufs=1) as sp, \
         tc.tile_pool(name="ps1", bufs=2, space="PSUM") as ps1, \
         tc.tile_pool(name="ps2", bufs=2, space="PSUM") as ps2:
        tg = wp.tile([1, Cm], bf16)
        tb = wp.tile([1, Cm], bf16)
        nc.gpsimd.dma_start(out=tg, in_=lng[:, :])
        nc.gpsimd.dma_start(out=tb, in_=lnb[:, :])
        twa = wp.tile([Cm, 128], bf16)
        twb = wp.tile([Cm, 128], bf16)
        nc.gpsimd.dma_start(out=twa, in_=wfin[:, 0:128])
        nc.gpsimd.dma_start(out=twb, in_=wfin[:, 128:256])
        tbf = wp.tile([128, 2], f32)
        nc.sync.dma_start(out=tbf, in_=bfin[:, :])
        tone = wp.tile([Cm, 1], bf16)
        nc.gpsimd.dma_start(out=tone, in_=ones128[:, :])
        tonesrow = wp.tile([1, LH], bf16)
        nc.gpsimd.dma_start(out=tonesrow, in_=onesrow[:, :])
        teps = wp.tile([128, 1], f32)
        nc.vector.memset(teps, 1e-5)

        tm0 = dp.tile([Cm, LH], bf16)
        tm1 = dp.tile([Cm, LH], bf16)
        tz = dp.tile([Cm, LH], bf16)
        for hh in range(2):
            s = slice(hh * LH // 2, (hh + 1) * LH // 2)
            nc.sync.dma_start(out=tm0[:, s], in_=m02[:, s])
            nc.sync.dma_start(out=tm1[:, s], in_=m13[:, s])
            nc.sync.dma_start(out=tz[:, s], in_=z_in[:, s])

        ty = dp.tile([Cm, LH], bf16)
        tmu = sp.tile([1, LH], bf16, tag="mu")
        tss = sp.tile([1, LH], bf16, tag="ss")
        for ci in range(NC3):
            sl = slice(ci * CH, (ci + 1) * CH)
            nc.vector.tensor_tensor(out=ty[:, sl], in0=tm0[:, sl],
                                    in1=tm1[:, sl], op=ADD)
            tsq = cp.tile([Cm, CH], bf16, tag="sq")
            nc.vector.tensor_tensor(out=tsq, in0=ty[:, sl], in1=ty[:, sl], op=MULT)
            for j in range(CH // 512):
                s2 = slice(j * 512, (j + 1) * 512)
                s2g = slice(ci * CH + j * 512, ci * CH + (j + 1) * 512)
                pmu = ps2.tile([1, 512], f32, tag="da")
                pss = ps2.tile([1, 512], f32, tag="db")
                nc.tensor.matmul(out=pmu[:, :], lhsT=tone[:, :],
                                 rhs=ty[:, s2g], start=True, stop=True)
                nc.tensor.matmul(out=pss[:, :], lhsT=tone[:, :],
                                 rhs=tsq[:, s2], start=True, stop=True)
                nc.scalar.activation(out=tmu[:, s2g], in_=pmu[:, :],
                                     func=AF.Identity, bias=0.0, scale=1.0)
                nc.scalar.activation(out=tss[:, s2g], in_=pss[:, :],
                                     func=AF.Identity, bias=0.0, scale=1.0)

        tmu2 = sp.tile([128, QL], f32, tag="r1")
        tss2 = sp.tile([128, QL], f32, tag="r2")
        nc.gpsimd.dma_start(out=tmu2[:, :], in_=tmu[0:1, :])
        nc.gpsimd.dma_start(out=tss2[:, :], in_=tss[0:1, :])
        tvar = sp.tile([128, QL], f32, tag="r3")
        nc.vector.tensor_tensor(out=tvar, in0=tmu2, in1=tmu2, op=MULT)
        nc.vector.tensor_tensor(out=tvar, in0=tss2, in1=tvar, op=SUB)
        tlnv = sp.tile([128, QL], f32, tag="r4")
        nc.scalar.activation(out=tlnv, in_=tvar, func=AF.Ln,
                             bias=teps[:, :], scale=1.0)
        trst = sp.tile([128, QL], f32, tag="r5")
        nc.scalar.activation(out=trst, in_=tlnv, func=AF.Exp,
                             bias=0.0, scale=-0.5)
        tmr = sp.tile([128, QL], f32, tag="r6")
        nc.vector.tensor_tensor(out=tmr, in0=tmu2, in1=trst, op=MULT)
        nc.vector.tensor_scalar_mul(out=tmr, in0=tmr, scalar1=-1.0)
        trow_r = sp.tile([1, LH], bf16, tag="r7")
        trow_m = sp.tile([1, LH], bf16, tag="r8")
        nc.gpsimd.dma_start(out=trow_r[0:1, :], in_=trst[:, :])
        nc.gpsimd.dma_start(out=trow_m[0:1, :], in_=tmr[:, :])

        for ci in range(NC3):
            for j in range(CH // 512):
                s2g = slice(ci * CH + j * 512, ci * CH + (j + 1) * 512)
                pR = ps1.tile([Cm, 512], f32, tag="R")
                pS = ps1.tile([Cm, 512], f32, tag="S")
                nc.tensor.matmul(out=pR[:, :], lhsT=tg[:, :],
                                 rhs=trow_r[:, s2g], start=True, stop=True)
                nc.tensor.matmul(out=pS[:, :], lhsT=tg[:, :],
                                 rhs=trow_m[:, s2g], start=True, stop=False)
                nc.tensor.matmul(out=pS[:, :], lhsT=tb[:, :],
                                 rhs=tonesrow[:, s2g], start=False, stop=True)
                tt = cp.tile([Cm, 512], bf16, tag="t")
                nc.vector.tensor_tensor(out=tt, in0=ty[:, s2g], in1=pR[:, :], op=MULT)
                nc.vector.tensor_tensor(out=tt, in0=tt, in1=pS[:, :], op=ADD)
                nc.vector.tensor_tensor(out=tt, in0=tt, in1=tz[:, s2g], op=MULT)
                pda = ps2.tile([128, 512], f32, tag="da")
                pdb = ps2.tile([128, 512], f32, tag="db")
                nc.tensor.matmul(out=pda[:, :], lhsT=twa[:, :], rhs=tt[:, :],
                                 start=True, stop=True)
                nc.tensor.matmul(out=pdb[:, :], lhsT=twb[:, :], rhs=tt[:, :],
                                 start=True, stop=True)
                tda = cp.tile([128, 512], f32, tag="oa")
                tdb = cp.tile([128, 512], f32, tag="ob")
                nc.scalar.activation(out=tda, in_=pda[:, :], func=AF.Identity,
                                     bias=tbf[:, 0:1], scale=1.0)
                nc.scalar.activation(out=tdb, in_=pdb[:, :], func=AF.Identity,
                                     bias=tbf[:, 1:2], scale=1.0)
                nc.sync.dma_start(out=d_out[0:128, s2g], in_=tda)
                nc.sync.dma_start(out=d_out[128:256, s2g], in_=tdb)
    return nc


# ------------------------------------------------------------------- host
def _get_ncs():
    if "ncs" not in _CACHE:
        nc1, nc2, nc3 = build_l1(), build_l2(), build_l3()
        for n in (nc1, nc2, nc3):
            _split_multiwaits(n)
        _CACHE["ncs"] = (nc1, nc2, nc3)
    return _CACHE["ncs"]


def kernel(x, cv1_w, cv1_b, scale_w, in_proj_w, conv_w, conv_b, x_proj_w,
           dt_w, dt_b, A_logs, Ds, ln_g, ln_b, out_proj_w, cv2_w, cv2_b):
    f = np.float32
    x = np.asarray(x, f)
    cv1_w = np.asarray(cv1_w, f); cv1_b = np.asarray(cv1_b, f)
    in_proj_w = np.asarray(in_proj_w, f)
    conv_w = np.asarray(conv_w, f); conv_b = np.asarray(conv_b, f)
    x_proj_w = np.asarray(x_proj_w, f)
    dt_w = np.asarray(dt_w, f); dt_b = np.asarray(dt_b, f)
    A_logs = np.asarray(A_logs, f); Ds = np.asarray(Ds, f)
    ln_g = np.asarray(ln_g, f); ln_b = np.asarray(ln_b, f)
    out_proj_w = np.asarray(out_proj_w, f)
    cv2_w = np.asarray(cv2_w, f); cv2_b = np.asarray(cv2_b, f)
    scale_v = np.asarray(scale_w, f).reshape(Cm)

    Wip_x, Wip_z = in_proj_w[:Cm], in_proj_w[Cm:]
    dwk = conv_w[:, 0]
    A = -np.exp(A_logs).reshape(K, Cm)
    Dk = Ds.reshape(K, Cm)
    W_dtk = np.einsum('kdr,krc->kdc', dt_w, x_proj_w[:, :R])
    WB, WC = x_proj_w[:, R], x_proj_w[:, R + 1]
    W_final = cv2_w @ (scale_v[:, None] * out_proj_w)

    # fold lhsT: (tap, k=h-chan, m=out-chan) -> host layout (k, tap, m)
    Wfold = np.einsum('cyx,cd->yxdc', dwk, Wip_x)      # (3,3, in, out)
    wfold_rm = np.ascontiguousarray(
        Wfold.reshape(9, Cm, Cm).transpose(1, 0, 2))   # row-major cores
    wbc_l = np.stack([WB[0], WC[0], WB[2], WC[2],
                      WB[1], WC[1], WB[3], WC[3]], axis=1)

    nc1, nc2, nc3 = _get_ncs()

    # ---------------- L1 ----------------
    l1_maps = []
    for core in range(8):
        b, half = core // 2, core % 2
        r0 = half * HH
        xs = np.zeros((C1, HH + 2, W), np.float32)
        lo, hi = r0 - 1, r0 + HH + 1
        slo, shi = max(lo, 0), min(hi, H)
        xs[:, slo - lo: shi - lo, :] = x[b, :, slo:shi, :]
        mask = np.ones((Cm, 2), np.float32)
        mask[:, 0] = 0.0 if half == 0 else 1.0
        mask[:, 1] = 1.0 if half == 0 else 0.0
        l1_maps.append({
            "x_in": xs,
            "wcv1": np.ascontiguousarray(cv1_w.T),
            "bcv1": cv1_b.reshape(Cm, 1),
            "wfold": wfold_rm,
            "bconv": conv_b.reshape(Cm, 1),
            "wz": np.ascontiguousarray(Wip_z.T),
            "wbc": np.ascontiguousarray(wbc_l),
            "hmask": mask,
        })
    r1 = _run(nc1, l1_maps, "L1")

    xc = np.zeros((B, Cm, L), NBF)
    zf = np.zeros((B, Cm, L), NBF)
    rows = np.zeros((B, 8, L), NBF)
    for core in range(8):
        b, half = core // 2, core % 2
        sl = slice(half * LH, (half + 1) * LH)
        xc[b][:, sl] = r1[core]["xc_out"]
        zf[b][:, sl] = r1[core]["z_out"]
        rows[b][:, sl] = r1[core]["bcr_out"]

    # ---------------- L2 ----------------
    def t_spatial(a):
        return np.ascontiguousarray(
            a.reshape(*a.shape[:-1], H, W).swapaxes(-1, -2).reshape(*a.shape[:-1], L))

    ident = np.eye(Cm, dtype=np.float32)
    l2_maps = []
    for core in range(8):
        b, g = core // 2, core % 2
        if g == 0:
            u = xc[b]
            kf, kr = 0, 2
            br_f, cr_f = rows[b][0], rows[b][1]
            br_r, cr_r = rows[b][2], rows[b][3]
        else:
            u = t_spatial(xc[b])
            kf, kr = 1, 3
            br_f, cr_f = t_spatial(rows[b][4]), t_spatial(rows[b][5])
            br_r, cr_r = t_spatial(rows[b][6]), t_spatial(rows[b][7])
        dsum_v = (Dk[kf] + Dk[kr]).astype(np.float32)
        l2_maps.append({
            "u_in": np.ascontiguousarray(u),
            "wdt_f": np.ascontiguousarray(W_dtk[kf].T),
            "wdt_r": np.ascontiguousarray(W_dtk[kr].T),
            "dtb_f": dt_b[kf].reshape(Cm, 1), "dtb_r": dt_b[kr].reshape(Cm, 1),
            "a_f": A[kf].reshape(Cm, 1).astype(f), "a_r": A[kr].reshape(Cm, 1).astype(f),
            "brow_f": br_f.reshape(1, L), "brow_r": br_r.reshape(1, L),
            "crow_f": cr_f.reshape(1, L), "crow_r": cr_r.reshape(1, L),
            "ident": ident, "diagd": np.diag(dsum_v).astype(np.float32),
        })
    r2 = _run(nc2, l2_maps, "L2")

    # ---------------- L3 ----------------
    l3_maps = []
    for b in range(B):
        m02 = r2[2 * b]["m_out"]
        m13t = t_spatial(r2[2 * b + 1]["m_out"])
        for half in range(2):
            sl = slice(half * LH, (half + 1) * LH)
            l3_maps.append({
                "m02": np.ascontiguousarray(m02[:, sl]),
                "m13": np.ascontiguousarray(m13t[:, sl]),
                "z_in": np.ascontiguousarray(zf[b][:, sl]),
                "lng": ln_g.reshape(1, Cm),
                "lnb": ln_b.reshape(1, Cm),
                "wfin": np.ascontiguousarray(W_final.T),
                "bfin": np.ascontiguousarray(cv2_b.reshape(2, 128).T),
                "ones128": np.full((Cm, 1), 1.0 / Cm, np.float32),
                "onesrow": np.ones((1, LH), np.float32),
            })
    r3 = _run(nc3, l3_maps, "L3")

    out = np.empty((B, C2, H, W), np.float32)
    for core in range(8):
        b, half = core // 2, core % 2
        sl = slice(half * LH, (half + 1) * LH)
        out[b].reshape(C2, L)[:, sl] = r3[core]["d_out"]
    out += x
    return out



# revision 10
# speedup vs baseline: 1.2648x; 1.0782x over previous
"""BottleneckMamba Trainium2 kernel (self-contained), v2.

out = x + cv2( scale * out_proj( LN(cross-merge(4-dir selective scan(N=1))) * z ) )

3 SPMD launches on 8 NeuronCores:
  L1 (core=(b, image-half)): cv1 -> h (bias on DVE); depthwise3x3*in_proj
     folded into 9 matmuls -> silu -> xc ; z = silu(Wz@h).
  L2 (core=(b, dir-group)): per direction: dtd = dt_w8 @ dts (rank-8 rows
     preshipped) -> exp/ln1p/exp on ACT (one table set) -> tbt = dt*v
     (v = u*B preshipped) -> tensor_tensor_scan -> mc = h*C (C broadcast by
     replicating DMA). Chunks processed ping-pong (reverse dir from the top,
     forward from the bottom) so both scans stream concurrently; m = mcf+mcr.
  L3 (core=(b, half)): y = m02 + m13^T + (sum_k D_k).xc, LayerNorm via
     matmul stats + rank-1 broadcast matmuls, (.*g + b) fused in STT with z,
     fused (cv2 @ diag(scale) @ out_proj) matmul -> bf16 delta.
Host: shards/reassembles, transposes between launches, computes the rank-8
dt rows + B/C rows + v = u*B from xc, adds residual x and cv2 bias.
"""
import os
import sys

sys.path.insert(0, '/opt/trn_rl_repo')

import numpy as np
import ml_dtypes

import concourse.bass as bass
import concourse.tile as tile
import concourse.mybir as mybir
from concourse.bass_utils import run_bass_kernel_spmd

bf16 = mybir.dt.bfloat16
f32 = mybir.dt.float32
MULT, ADD = mybir.AluOpType.mult, mybir.AluOpType.add
SUB = mybir.AluOpType.subtract
AF = mybir.ActivationFunctionType
NBF = ml_dtypes.bfloat16

B, C1, C2, H, W = 4, 256, 256, 128, 128
Cm, K, R = 128, 4, 8
L = H * W          # 16384
HH = H // 2        # 64 rows per half
LH = HH * W        # 8192
CH = 2048          # L2 chunk
NCH = L // CH      # 8
CH3 = 2048         # L3 chunk
NC3 = LH // CH3    # 4

EXEC_TIMES = {}    # launch -> exec ns (MAMBA_TRACE=1)
TRACES = {}        # launch -> (insts, trace_path) (MAMBA_TRACE=1)
_CACHE = {}


def _split_multiwaits(nc):
    """walrus here accepts ONE sync-wait per instruction; hoist extras into
    single-wait same-engine NOPs inserted before the instruction."""
    for f in nc.m.functions:
        for bb in f.blocks:
            il = bb.instructions
            i = 0
            while i < len(il):
                ins = il[i]
                si = getattr(ins, "sync_info", None)
                if si is not None and len(si.on_wait) > 1:
                    waits = list(si.on_wait)
                    ins.sync_info = mybir.SyncInfo(
                        on_wait=[waits[-1]], on_update=list(si.on_update))
                    for w in waits[:-1]:
                        nop = mybir.InstNoOp(
                            name=nc.get_next_instruction_name(), ins=[], outs=[])
                        nop.engine = ins.engine
                        nop.sync_info = mybir.SyncInfo(on_wait=[w], on_update=[])
                        nc.register_instruction(nop, overwrite=True)
                        il.insert(i, nop)
                        i += 1
                i += 1


def _new_nc():
    return bass.Bass("TRN2", target_bir_lowering=False, debug=False,
                     enable_asserts=True, num_devices=8)


def _run(nc, in_maps, name):
    trace = os.environ.get("MAMBA_TRACE", "0") == "1"
    res = run_bass_kernel_spmd(nc, in_maps, core_ids=list(range(8)), trace=trace)
    if trace:
        EXEC_TIMES[name] = res.exec_time_ns
        TRACES[name] = res.instructions_and_trace
    return res.results


# ------------------------------------------------------------------- L1
def build_l1():
    nc = _new_nc()
    x_in = nc.dram_tensor("x_in", [C1, HH + 2, W], bf16, kind="ExternalInput")
    wcv1 = nc.dram_tensor("wcv1", [C1, Cm], f32, kind="ExternalInput")       # lhsT
    bcv1 = nc.dram_tensor("bcv1", [Cm, 1], f32, kind="ExternalInput")
    wfold = nc.dram_tensor("wfold", [Cm, 9, Cm], f32, kind="ExternalInput")  # (k, tap, m)
    bconv = nc.dram_tensor("bconv", [Cm, 1], f32, kind="ExternalInput")
    wz = nc.dram_tensor("wz", [Cm, Cm], f32, kind="ExternalInput")           # lhsT
    hmask = nc.dram_tensor("hmask", [Cm, 2], f32, kind="ExternalInput")
    xc_out = nc.dram_tensor("xc_out", [Cm, LH], bf16, kind="ExternalOutput")
    z_out = nc.dram_tensor("z_out", [Cm, LH], bf16, kind="ExternalOutput")

    HP = HH + 2   # 66
    WP = W + 2    # 130

    with tile.TileContext(nc) as tc, \
         tc.tile_pool(name="w", bufs=1) as wp, \
         tc.tile_pool(name="d", bufs=1) as dp, \
         tc.tile_pool(name="ps", bufs=2, space="PSUM") as pp:
        tw1a = wp.tile([128, Cm], bf16)
        tw1b = wp.tile([128, Cm], bf16)
        nc.gpsimd.dma_start(out=tw1a, in_=wcv1[0:128, :])
        nc.gpsimd.dma_start(out=tw1b, in_=wcv1[128:256, :])
        twf = wp.tile([Cm, 9, Cm], bf16)
        nc.gpsimd.dma_start(out=twf, in_=wfold[:, :, :])
        twz = wp.tile([Cm, Cm], bf16)
        nc.gpsimd.dma_start(out=twz, in_=wz[:, :])
        tb1 = wp.tile([Cm, 1], f32)
        nc.scalar.dma_start(out=tb1, in_=bcv1[:, :])
        tbc = wp.tile([Cm, 1], f32)
        nc.scalar.dma_start(out=tbc, in_=bconv[:, :])
        tmask = wp.tile([Cm, 2], f32)
        nc.scalar.dma_start(out=tmask, in_=hmask[:, :])

        txa = dp.tile([128, HP, W], bf16)
        txb = dp.tile([128, HP, W], bf16)
        for rb in range(0, HP, 11):
            nc.gpsimd.dma_start(out=txa[:, rb:rb + 11, :], in_=x_in[0:128, rb:rb + 11, :])
            nc.sync.dma_start(out=txb[:, rb:rb + 11, :], in_=x_in[128:256, rb:rb + 11, :])

        th = dp.tile([Cm, HP, WP], bf16)
        nc.vector.memset(th[:, :, 0:1], 0.0)
        nc.vector.memset(th[:, :, WP - 1:WP], 0.0)

        # cv1 over 66 rows: 16 chunks of 4 rows + 1 chunk of 2 rows
        row_chunks = [(r0, 4) for r0 in range(0, 64, 4)] + [(64, 2)]
        for r0, nr in row_chunks:
            pt = pp.tile([Cm, 512], f32, tag="cv1")
            nn = nr * W
            nc.tensor.matmul(out=pt[:, :nn], lhsT=tw1a[:, :],
                             rhs=txa[:, r0:r0 + nr, :], start=True, stop=False)
            nc.tensor.matmul(out=pt[:, :nn], lhsT=tw1b[:, :],
                             rhs=txb[:, r0:r0 + nr, :], start=False, stop=True)
            nc.vector.tensor_scalar_add(out=th[:, r0:r0 + nr, 1:W + 1],
                                        in0=pt[:, :nn], scalar1=tb1[:, 0:1])
        nc.vector.tensor_scalar_mul(out=th[:, 0, :], in0=th[:, 0, :],
                                    scalar1=tmask[:, 0:1])
        nc.vector.tensor_scalar_mul(out=th[:, HP - 1, :], in0=th[:, HP - 1, :],
                                    scalar1=tmask[:, 1:2])

        txc = dp.tile([Cm, HH, W], bf16)
        tz = dp.tile([Cm, HH, W], bf16)
        for r0 in range(0, HH, 4):
            pt = pp.tile([Cm, 512], f32, tag="fold")
            for t in range(9):
                dy, dx = t // 3 - 1, t % 3 - 1
                nc.tensor.matmul(
                    out=pt[:, :], lhsT=twf[:, t, :],
                    rhs=th[:, r0 + 1 + dy:r0 + 5 + dy, 1 + dx:W + 1 + dx],
                    start=(t == 0), stop=(t == 8))
            nc.scalar.activation(out=txc[:, r0:r0 + 4, :], in_=pt[:, :],
                                 func=AF.Silu, bias=tbc[:, :], scale=1.0)
            ptz = pp.tile([Cm, 512], f32, tag="z")
            nc.tensor.matmul(out=ptz[:, :], lhsT=twz[:, :],
                             rhs=th[:, r0 + 1:r0 + 5, 1:W + 1],
                             start=True, stop=True)
            nc.scalar.activation(out=tz[:, r0:r0 + 4, :], in_=ptz[:, :],
                                 func=AF.Silu, bias=0.0, scale=1.0)

            if r0 % 16 == 12:  # flush every 16 rows
                rs = r0 - 12
                nc.gpsimd.dma_start(out=xc_out[:, rs * W:(r0 + 4) * W],
                                    in_=txc[:, rs:r0 + 4, :])
                nc.sync.dma_start(out=z_out[:, rs * W:(r0 + 4) * W],
                                  in_=tz[:, rs:r0 + 4, :])
    return nc


# ------------------------------------------------------------------- L2
def build_l2():
    nc = _new_nc()
    dts_f = nc.dram_tensor("dts_f", [R, L], bf16, kind="ExternalInput")
    dts_r = nc.dram_tensor("dts_r", [R, L], bf16, kind="ExternalInput")
    v_f = nc.dram_tensor("v_f", [Cm, L], bf16, kind="ExternalInput")
    v_r = nc.dram_tensor("v_r", [Cm, L], bf16, kind="ExternalInput")
    crow_f = nc.dram_tensor("crow_f", [1, L], bf16, kind="ExternalInput")
    crow_r = nc.dram_tensor("crow_r", [1, L], bf16, kind="ExternalInput")
    wdt_f = nc.dram_tensor("wdt_f", [R, Cm], f32, kind="ExternalInput")   # lhsT
    wdt_r = nc.dram_tensor("wdt_r", [R, Cm], f32, kind="ExternalInput")
    dtb_f = nc.dram_tensor("dtb_f", [Cm, 1], f32, kind="ExternalInput")
    dtb_r = nc.dram_tensor("dtb_r", [Cm, 1], f32, kind="ExternalInput")
    a_f = nc.dram_tensor("a_f", [Cm, 1], f32, kind="ExternalInput")
    a_r = nc.dram_tensor("a_r", [Cm, 1], f32, kind="ExternalInput")
    m_out = nc.dram_tensor("m_out", [Cm, L], bf16, kind="ExternalOutput")

    def bc_ap(t, sl):  # DRAM row slice -> partition-replicated AP
        return bass.AP(tensor=t, offset=sl.start, ap=[[0, 128], [1, sl.stop - sl.start]])

    with tile.TileContext(nc) as tc, \
         tc.tile_pool(name="w", bufs=1) as wp, \
         tc.tile_pool(name="full", bufs=1) as fp, \
         tc.tile_pool(name="ds", bufs=2) as dsp, \
         tc.tile_pool(name="vv", bufs=2) as vp, \
         tc.tile_pool(name="ck", bufs=2) as cp, \
         tc.tile_pool(name="hk", bufs=2) as hp, \
         tc.tile_pool(name="bc", bufs=2) as bp, \
         tc.tile_pool(name="mm", bufs=2) as mp, \
         tc.tile_pool(name="psd", bufs=2, space="PSUM") as psd:
        twf_ = wp.tile([R, Cm], bf16)
        twr_ = wp.tile([R, Cm], bf16)
        nc.gpsimd.dma_start(out=twr_, in_=wdt_r[:, :])
        nc.gpsimd.dma_start(out=twf_, in_=wdt_f[:, :])
        tbf = wp.tile([Cm, 1], f32)
        tbr = wp.tile([Cm, 1], f32)
        taf = wp.tile([Cm, 1], f32)
        tar = wp.tile([Cm, 1], f32)
        nc.gpsimd.dma_start(out=tbf, in_=dtb_f[:, :])
        nc.gpsimd.dma_start(out=tbr, in_=dtb_r[:, :])
        nc.gpsimd.dma_start(out=taf, in_=a_f[:, :])
        nc.gpsimd.dma_start(out=tar, in_=a_r[:, :])

        tmcf = fp.tile([Cm, L], bf16)   # h_f * C_f, natural position order
        tmcr = fp.tile([Cm, L], bf16)   # h_r * C_r, natural position order

        state = {"prev_f": None, "prev_r": None}

        def side(ci, rev):
            sl = slice(ci * CH, (ci + 1) * CH)
            sfx = "r" if rev else "f"
            tdts, tv_d, tcr, tw, tb, ta = (
                (dts_r, v_r, crow_r, twr_, tbr, tar) if rev else
                (dts_f, v_f, crow_f, twf_, tbf, taf))
            td = dsp.tile([R, CH], bf16, tag="d" + sfx)
            nc.gpsimd.dma_start(out=td, in_=tdts[:, sl])
            tv = vp.tile([Cm, CH], bf16, tag="v" + sfx)
            nc.sync.dma_start(out=tv, in_=tv_d[:, sl])
            tcb = bp.tile([Cm, CH], bf16, tag="c" + sfx)
            nc.gpsimd.dma_start(out=tcb, in_=bc_ap(tcr, sl))

            pt = psd.tile([Cm, CH], f32, tag="dtd")
            for j in range(CH // 512):
                nc.tensor.matmul(
                    out=pt[:, j * 512:(j + 1) * 512], lhsT=tw[:, :],
                    rhs=td[:, j * 512:(j + 1) * 512], start=True, stop=True)
            te1 = cp.tile([Cm, CH], bf16, tag="e1")
            nc.scalar.activation(out=te1, in_=pt[:, :], func=AF.Exp,
                                 bias=tb[:, :], scale=1.0)
            tdt = cp.tile([Cm, CH], bf16, tag="dt")
            nc.scalar.activation(out=tdt, in_=te1, func=AF.Ln, bias=1.0, scale=1.0)
            tav = cp.tile([Cm, CH], bf16, tag="av")
            nc.scalar.activation(out=tav, in_=tdt, func=AF.Exp,
                                 bias=0.0, scale=ta[:, :])
            tbt = cp.tile([Cm, CH], bf16, tag="bt")
            nc.vector.tensor_tensor(out=tbt, in0=tdt, in1=tv, op=MULT)
            thc = hp.tile([Cm, CH], bf16, tag="h" + sfx)
            if rev:
                prev = state["prev_r"]
                nc.vector.tensor_tensor_scan(
                    out=thc[:, ::-1], data0=tav[:, ::-1], data1=tbt[:, ::-1],
                    initial=0.0 if prev is None else prev, op0=MULT, op1=ADD)
                state["prev_r"] = thc[:, 0:1]
                nc.vector.tensor_tensor(out=tmcr[:, sl], in0=thc, in1=tcb, op=MULT)
            else:
                prev = state["prev_f"]
                nc.vector.tensor_tensor_scan(
                    out=thc, data0=tav, data1=tbt,
                    initial=0.0 if prev is None else prev, op0=MULT, op1=ADD)
                state["prev_f"] = thc[:, CH - 1:CH]
                nc.vector.tensor_tensor(out=tmcf[:, sl], in0=thc, in1=tcb, op=MULT)

        for s in range(NCH):
            side(NCH - 1 - s, rev=True)
            side(s, rev=False)
            if s >= NCH // 2:
                for c in (s, NCH - 1 - s):
                    slc = slice(c * CH, (c + 1) * CH)
                    tm = mp.tile([Cm, CH], bf16, tag="m")
                    nc.vector.tensor_tensor(out=tm, in0=tmcf[:, slc],
                                            in1=tmcr[:, slc], op=ADD)
                    nc.sync.dma_start(out=m_out[:, slc], in_=tm)
    return nc


# ------------------------------------------------------------------- L3
def build_l3():
    nc = _new_nc()
    m02 = nc.dram_tensor("m02", [Cm, LH], bf16, kind="ExternalInput")
    m13 = nc.dram_tensor("m13", [Cm, LH], bf16, kind="ExternalInput")
    z_in = nc.dram_tensor("z_in", [Cm, LH], bf16, kind="ExternalInput")
    xc_in = nc.dram_tensor("xc_in", [Cm, LH], bf16, kind="ExternalInput")
    lng = nc.dram_tensor("lng", [1, Cm], f32, kind="ExternalInput")
    lnb_c = nc.dram_tensor("lnb_c", [Cm, 1], f32, kind="ExternalInput")
    dtot = nc.dram_tensor("dtot", [Cm, 1], f32, kind="ExternalInput")
    wfin = nc.dram_tensor("wfin", [Cm, C2], f32, kind="ExternalInput")
    ones128 = nc.dram_tensor("ones128", [Cm, 1], f32, kind="ExternalInput")
    d_out = nc.dram_tensor("d_out", [C2, LH], bf16, kind="ExternalOutput")

    QL = LH // 128  # 64

    with tile.TileContext(nc) as tc, \
         tc.tile_pool(name="w", bufs=1) as wp, \
         tc.tile_pool(name="d", bufs=1) as dp, \
         tc.tile_pool(name="c", bufs=3) as cp, \
         tc.tile_pool(name="st", bufs=1) as sp, \
         tc.tile_pool(name="ps1", bufs=2, space="PSUM") as ps1, \
         tc.tile_pool(name="ps2", bufs=1, space="PSUM") as ps2, \
         tc.tile_pool(name="ps3", bufs=2, space="PSUM") as ps3:
        tg = wp.tile([1, Cm], bf16)
        nc.gpsimd.dma_start(out=tg, in_=lng[:, :])
        twa = wp.tile([Cm, 128], bf16)
        twb = wp.tile([Cm, 128], bf16)
        nc.gpsimd.dma_start(out=twa, in_=wfin[:, 0:128])
        nc.gpsimd.dma_start(out=twb, in_=wfin[:, 128:256])
        tone = wp.tile([Cm, 1], bf16)
        nc.gpsimd.dma_start(out=tone, in_=ones128[:, :])
        tlnb = wp.tile([Cm, 1], f32)
        nc.scalar.dma_start(out=tlnb, in_=lnb_c[:, :])
        tdt = wp.tile([Cm, 1], f32)
        nc.scalar.dma_start(out=tdt, in_=dtot[:, :])
        teps = wp.tile([128, 1], f32)
        nc.vector.memset(teps, 1e-5)

        tm0 = dp.tile([Cm, LH], bf16)
        tm1 = dp.tile([Cm, LH], bf16)
        tz = dp.tile([Cm, LH], bf16)
        txc = dp.tile([Cm, LH], bf16)
        for hh in range(2):
            s = slice(hh * LH // 2, (hh + 1) * LH // 2)
            nc.gpsimd.dma_start(out=tm0[:, s], in_=m02[:, s])
            nc.gpsimd.dma_start(out=tm1[:, s], in_=m13[:, s])
            nc.sync.dma_start(out=tz[:, s], in_=z_in[:, s])
            nc.sync.dma_start(out=txc[:, s], in_=xc_in[:, s])

        ty = dp.tile([Cm, LH], bf16)        # y = m0 + m1 + Dtot*xc
        trow2 = sp.tile([33, LH], bf16)     # partition 0: mean, partition 32: meansq
        for ci in range(NC3):
            sl = slice(ci * CH3, (ci + 1) * CH3)
            tdx = cp.tile([Cm, CH3], bf16, tag="dx")
            nc.scalar.activation(out=tdx, in_=txc[:, sl], func=AF.Identity,
                                 bias=0.0, scale=tdt[:, :])
            ta_ = cp.tile([Cm, CH3], bf16, tag="tya")
            nc.vector.tensor_tensor(out=ta_, in0=tm0[:, sl], in1=tm1[:, sl], op=ADD)
            nc.vector.tensor_tensor(out=ty[:, sl], in0=ta_, in1=tdx, op=ADD)
            tsq = cp.tile([Cm, CH3], bf16, tag="sq")
            nc.vector.tensor_tensor(out=tsq, in0=ty[:, sl], in1=ty[:, sl], op=MULT)
            for j in range(CH3 // 512):
                s2 = slice(j * 512, (j + 1) * 512)
                s2g = slice(ci * CH3 + j * 512, ci * CH3 + (j + 1) * 512)
                pst = ps1.tile([33, 512], f32, tag="st")
                nc.tensor.matmul(out=pst[0:1, :], lhsT=tone[:, :],
                                 rhs=ty[:, s2g], start=True, stop=True)
                nc.tensor.matmul(out=pst[32:33, :], lhsT=tone[:, :],
                                 rhs=tsq[:, s2], start=True, stop=True)
                nc.scalar.activation(out=trow2[:, s2g], in_=pst[:, :],
                                     func=AF.Identity, bias=0.0, scale=1.0)

        tmu2 = sp.tile([128, QL], f32, tag="r1")
        tss2 = sp.tile([128, QL], f32, tag="r2")
        nc.gpsimd.dma_start(out=tmu2[:, :], in_=trow2[0:1, :])
        nc.gpsimd.dma_start(out=tss2[:, :], in_=trow2[32:33, :])
        tvar = sp.tile([128, QL], f32, tag="r3")
        nc.vector.tensor_tensor(out=tvar, in0=tmu2, in1=tmu2, op=MULT)
        nc.vector.tensor_tensor(out=tvar, in0=tss2, in1=tvar, op=SUB)
        tlnv = sp.tile([128, QL], f32, tag="r4")
        nc.scalar.activation(out=tlnv, in_=tvar, func=AF.Ln,
                             bias=teps[:, :], scale=1.0)
        trst = sp.tile([128, QL], f32, tag="r5")
        nc.scalar.activation(out=trst, in_=tlnv, func=AF.Exp,
                             bias=0.0, scale=-0.5)
        tmr = sp.tile([128, QL], f32, tag="r6")
        nc.vector.tensor_tensor(out=tmr, in0=tmu2, in1=trst, op=MULT)
        nc.vector.tensor_scalar_mul(out=tmr, in0=tmr, scalar1=-1.0)
        trow_r = sp.tile([1, LH], bf16, tag="r7")
        trow_m = sp.tile([1, LH], bf16, tag="r8")
        nc.gpsimd.dma_start(out=trow_r[0:1, :], in_=trst[:, :])
        nc.gpsimd.dma_start(out=trow_m[0:1, :], in_=tmr[:, :])

        for q in range(LH // 512):
            s2g = slice(q * 512, (q + 1) * 512)
            prs = ps2.tile([Cm, 1024], f32, tag="rs")
            nc.tensor.matmul(out=prs[:, 0:512], lhsT=tg[:, :],
                             rhs=trow_r[:, s2g], start=True, stop=True)
            nc.tensor.matmul(out=prs[:, 512:1024], lhsT=tg[:, :],
                             rhs=trow_m[:, s2g], start=True, stop=True)
            tR = cp.tile([Cm, 512], bf16, tag="cR")
            nc.scalar.activation(out=tR, in_=prs[:, 0:512], func=AF.Identity,
                                 bias=0.0, scale=1.0)
            tS = cp.tile([Cm, 512], bf16, tag="cS")
            nc.scalar.activation(out=tS, in_=prs[:, 512:1024], func=AF.Identity,
                                 bias=tlnb[:, :], scale=1.0)
            tt1 = cp.tile([Cm, 512], bf16, tag="t1")
            nc.vector.tensor_tensor(out=tt1, in0=ty[:, s2g], in1=tR, op=MULT)
            tt2 = cp.tile([Cm, 512], bf16, tag="t2")
            nc.vector.tensor_tensor(out=tt2, in0=tt1, in1=tS, op=ADD)
            tt3 = cp.tile([Cm, 512], bf16, tag="t3")
            nc.vector.tensor_tensor(out=tt3, in0=tt2, in1=tz[:, s2g], op=MULT)
            pd = ps3.tile([128, 1024], f32, tag="d")
            nc.tensor.matmul(out=pd[:, 0:512], lhsT=twa[:, :], rhs=tt3[:, :],
                             start=True, stop=True)
            nc.tensor.matmul(out=pd[:, 512:1024], lhsT=twb[:, :], rhs=tt3[:, :],
                             start=True, stop=True)
            td_ = cp.tile([128, 1024], bf16, tag="td")
            if q % 2 == 0:
                nc.scalar.activation(out=td_, in_=pd[:, :],
                                     func=AF.Identity, bias=0.0, scale=1.0)
            else:
                nc.vector.tensor_copy(out=td_, in_=pd[:, :])
            nc.gpsimd.dma_start(out=d_out[0:128, s2g], in_=td_[:, 0:512])
            nc.sync.dma_start(out=d_out[128:256, s2g], in_=td_[:, 512:1024])
    return nc


# ------------------------------------------------------------------- host
def _get_ncs():
    if "ncs" not in _CACHE:
        nc1, nc2, nc3 = build_l1(), build_l2(), build_l3()
        for n in (nc1, nc2, nc3):
            _split_multiwaits(n)
        _CACHE["ncs"] = (nc1, nc2, nc3)
    return _CACHE["ncs"]


def kernel(x, cv1_w, cv1_b, scale_w, in_proj_w, conv_w, conv_b, x_proj_w,
           dt_w, dt_b, A_logs, Ds, ln_g, ln_b, out_proj_w, cv2_w, cv2_b):
    f = np.float32
    x = np.asarray(x, f)
    cv1_w = np.asarray(cv1_w, f); cv1_b = np.asarray(cv1_b, f)
    in_proj_w = np.asarray(in_proj_w, f)
    conv_w = np.asarray(conv_w, f); conv_b = np.asarray(conv_b, f)
    x_proj_w = np.asarray(x_proj_w, f)
    dt_w = np.asarray(dt_w, f); dt_b = np.asarray(dt_b, f)
    A_logs = np.asarray(A_logs, f); Ds = np.asarray(Ds, f)
    ln_g = np.asarray(ln_g, f); ln_b = np.asarray(ln_b, f)
    out_proj_w = np.asarray(out_proj_w, f)
    cv2_w = np.asarray(cv2_w, f); cv2_b = np.asarray(cv2_b, f)
    scale_v = np.asarray(scale_w, f).reshape(Cm)

    Wip_x, Wip_z = in_proj_w[:Cm], in_proj_w[Cm:]
    dwk = conv_w[:, 0]
    A = -np.exp(A_logs).reshape(K, Cm)
    Dk = Ds.reshape(K, Cm)
    Dtot = Dk.sum(axis=0)                              # (Cm,)
    Wdts8 = x_proj_w[:, :R]                            # (K, R, Cm)
    WB, WC = x_proj_w[:, R], x_proj_w[:, R + 1]        # (K, Cm)
    W_final = cv2_w @ (scale_v[:, None] * out_proj_w)

    # fold lhsT: (tap, k=h-chan, m=out-chan) -> host layout (k, tap, m)
    Wfold = np.einsum('cyx,cd->yxdc', dwk, Wip_x)      # (3,3, in, out)
    wfold_rm = np.ascontiguousarray(
        Wfold.reshape(9, Cm, Cm).transpose(1, 0, 2))   # row-major cores

    nc1, nc2, nc3 = _get_ncs()

    # ---------------- L1 ----------------
    l1_maps = []
    for core in range(8):
        b, half = core // 2, core % 2
        r0 = half * HH
        xs = np.zeros((C1, HH + 2, W), NBF)
        lo, hi = r0 - 1, r0 + HH + 1
        slo, shi = max(lo, 0), min(hi, H)
        xs[:, slo - lo: shi - lo, :] = x[b, :, slo:shi, :].astype(NBF)
        mask = np.ones((Cm, 2), np.float32)
        mask[:, 0] = 0.0 if half == 0 else 1.0
        mask[:, 1] = 1.0 if half == 0 else 0.0
        l1_maps.append({
            "x_in": xs,
            "wcv1": np.ascontiguousarray(cv1_w.T),
            "bcv1": cv1_b.reshape(Cm, 1),
            "wfold": wfold_rm,
            "bconv": conv_b.reshape(Cm, 1),
            "wz": np.ascontiguousarray(Wip_z.T),
            "hmask": mask,
        })
    r1 = _run(nc1, l1_maps, "L1")

    xc = np.zeros((B, Cm, L), NBF)
    zf = np.zeros((B, Cm, L), NBF)
    for core in range(8):
        b, half = core // 2, core % 2
        sl = slice(half * LH, (half + 1) * LH)
        xc[b][:, sl] = r1[core]["xc_out"]
        zf[b][:, sl] = r1[core]["z_out"]

    # ---------------- L2 ----------------
    def t_spatial(a):
        return np.ascontiguousarray(
            a.reshape(*a.shape[:-1], H, W).swapaxes(-1, -2).reshape(*a.shape[:-1], L))

    l2_maps = []
    for core in range(8):
        b, g = core // 2, core % 2
        if g == 0:
            u = xc[b]
            kf, kr = 0, 2
        else:
            u = t_spatial(xc[b])
            kf, kr = 1, 3
        uf = u.astype(np.float32)
        m = {}
        for sfx, k in (("f", kf), ("r", kr)):
            m["dts_" + sfx] = (Wdts8[k] @ uf).astype(NBF)
            brow = WB[k] @ uf
            m["crow_" + sfx] = (WC[k] @ uf).astype(NBF).reshape(1, L)
            m["v_" + sfx] = (uf * brow[None, :]).astype(NBF)
            m["wdt_" + sfx] = np.ascontiguousarray(dt_w[k].T)      # (R, Cm)
            m["dtb_" + sfx] = dt_b[k].reshape(Cm, 1)
            m["a_" + sfx] = A[k].reshape(Cm, 1).astype(f)
        l2_maps.append(m)
    r2 = _run(nc2, l2_maps, "L2")

    # ---------------- L3 ----------------
    l3_maps = []
    for b in range(B):
        m02 = r2[2 * b]["m_out"]
        m13t = t_spatial(r2[2 * b + 1]["m_out"])
        for half in range(2):
            sl = slice(half * LH, (half + 1) * LH)
            l3_maps.append({
                "m02": np.ascontiguousarray(m02[:, sl]),
                "m13": np.ascontiguousarray(m13t[:, sl]),
                "z_in": np.ascontiguousarray(zf[b][:, sl]),
                "xc_in": np.ascontiguousarray(xc[b][:, sl]),
                "lng": ln_g.reshape(1, Cm),
                "lnb_c": ln_b.reshape(Cm, 1),
                "dtot": Dtot.reshape(Cm, 1),
                "wfin": np.ascontiguousarray(W_final.T),
                "ones128": np.full((Cm, 1), 1.0 / Cm, np.float32),
            })
    r3 = _run(nc3, l3_maps, "L3")

    out = np.empty((B, C2, H, W), np.float32)
    for core in range(8):
        b, half = core // 2, core % 2
        sl = slice(half * LH, (half + 1) * LH)
        out[b].reshape(C2, L)[:, sl] = r3[core]["d_out"]
    out += x
    out += cv2_b[None, :, None, None]
    return out
